# revision 14
# baseline (speedup 1.0000x reference)
"""Trainium2 Bass/Tile kernel for nn_CrossAttentionModule (4 cross-attention layers).

Sharding (8 cores): core c handles batch b = c//4 and head-pair p = c%4
(heads 2p, 2p+1)  -- data-parallel over batch, tensor-parallel over heads.
Each core computes q/k/v projections for its 128-wide slice of INNER, the
attention for its two heads, and a PARTIAL output projection (its heads'
contribution).  The host sums the 4 partials per batch and adds the output
bias.

Per-layer on-device dataflow (everything in "transposed" layout, so no
transposes are ever needed):
  xT (C, L) comes straight from HBM (ml_* tensors are (B, C, H, W)).
  qT (128, L)  = wq_slice^T @ xcT          (K=C, accumulated over C/128 chunks)
  kT (128, L)  = wk_slice^T @ xrT
  v  (L, 128)  = xfT^T @ wv_slice          (keys on partitions)
  scoresT tile (Lk, Lq) = kT_head^T-free @ qT_head  (K=64; the two heads live
     on disjoint partition ranges 0:64 / 64:128 so their score matmuls run
     concurrently in disjoint PE row-groups)
  probsT = exp(scoresT / 8)                (ScalarE, no max subtraction:
                                            scores are ~N(0,1) so exp is safe)
  attn_outT (65, Lq) += [v_head | ones]^T @ probsT   (ones column produces the
                                            softmax denominator in row 64)
  normalize: recip = 1/denom (DVE approx), broadcast across partitions via a
     K=1 PE outer-product, multiply.
  partial outT (C, Lq) = wo_h0^T @ attn0 + wo_h1^T @ attn1   (two K=64 matmuls)

Matmuls run in bf16 (fp32 matmul is 4x slower on TRN2 PE); accumulation is
fp32 in PSUM.  Softmax numerator/denominator use the same bf16 probs, so
rounding largely cancels in the ratio.
"""

import os
from contextlib import ExitStack

import ml_dtypes
import numpy as np

import concourse.bass as bass
import concourse.mybir as mybir
import concourse.tile as tile
from concourse.bass_utils import run_bass_kernel_spmd

BF16 = mybir.dt.bfloat16
FP32 = mybir.dt.float32
FP32R = mybir.dt.float32r
NP_BF16 = ml_dtypes.bfloat16

DIMS = [128, 256, 512, 512]
RES = [64, 32, 16, 8]
B = 2
DH = 64
N_CORES = 8

LAYERS = tuple(
    int(x) for x in os.environ.get("KERNEL_LAYERS", "0,1,2,3").split(",") if x != ""
)

_NC = None
LAST_RESULTS = None


def _emit_layer(tc, nc, pools, io, i, ones_sb):
    C, R = DIMS[i], RES[i]
    L = R * R
    KC = C // 128          # contraction chunks for projections
    CH = min(512, L)       # query-chunk width (free dim of PSUM tiles)
    NCH = L // CH
    LP = min(L, 128)       # key-tile partition size
    NT = L // LP           # number of key tiles

    p_x, p_w, p_qk, p_v, p_pr, p_ao, p_sm, p_out = (
        pools["x"], pools["w"], pools["qk"], pools["v"], pools["pr"],
        pools["ao"], pools["sm"], pools["out"],
    )
    ps_s, ps_pv, ps_p = pools["ps_s"], pools["ps_pv"], pools["ps_p"]

    # ---- load weights / biases ------------------------------------------
    wq_sb = p_w.tile([128, KC, 128], BF16, tag="wq")
    wk_sb = p_w.tile([128, KC, 128], BF16, tag="wk")
    wv_sb = p_w.tile([128, KC, 128], BF16, tag="wv")
    for w_sb, nm in ((wq_sb, "wq"), (wk_sb, "wk"), (wv_sb, "wv")):
        nc.sync.dma_start(
            w_sb[:], io[f"{nm}{i}"][:].rearrange("(kc p) m -> p kc m", p=128)
        )
    wo0_sb = p_w.tile([64, C], BF16, tag="wo0")   # rows 0:64  of wo slice (head 0 dims)
    wo1_sb = p_w.tile([64, C], BF16, tag="wo1")   # rows 64:128 (head 1 dims)
    nc.sync.dma_start(wo0_sb[:], io[f"wo{i}"][0:64, :])
    nc.sync.dma_start(wo1_sb[:], io[f"wo{i}"][64:128, :])
    bq_sb = p_w.tile([128, 1], FP32, tag="bq")
    bk_sb = p_w.tile([128, 1], FP32, tag="bk")
    nc.sync.dma_start(bq_sb[:], io[f"bq{i}"][:])
    nc.sync.dma_start(bk_sb[:], io[f"bk{i}"][:])
    bv_sb = p_w.tile([1, 128], FP32, tag="bv")
    nc.sync.dma_start(bv_sb[:], io[f"bv{i}"][:])
    # broadcast bv across partitions with a K=1 outer product: bvb[m, n] = bv[n]
    # (fp32r bitcast: full-rate streaming vs 4 cycles/row for plain fp32)
    bvb_ps = ps_p.tile([128, 128], FP32, tag="pp")
    nc.tensor.matmul(bvb_ps[:], lhsT=ones_sb[:, 0:128].bitcast(FP32R),
                     rhs=bv_sb[:].bitcast(FP32R), start=True, stop=True)
    bv_bc = p_w.tile([128, 128], FP32, tag="bvbc")
    nc.vector.tensor_copy(bv_bc[:], bvb_ps[:])

    # ---- load activations (already bf16, already transposed as (C, L)) ---
    # Chunked DMAs so the first projections / scores never wait on the whole
    # tensor transfer.
    xc_sb = p_x.tile([128, KC, L], BF16, tag="xc")
    xr_sb = p_x.tile([128, KC, L], BF16, tag="xr")
    xf_sb = p_x.tile([128, KC, L], BF16, tag="xf")
    x_dram = {
        nm: io[f"{nm}{i}"][:].rearrange("(kc p) l -> p kc l", p=128)
        for nm in ("xc", "xr", "xf")
    }
    for ch in range(NCH):
        chs = slice(ch * CH, (ch + 1) * CH)
        for x_sb, nm in ((xc_sb, "xc"), (xr_sb, "xr"), (xf_sb, "xf")):
            nc.sync.dma_start(x_sb[:, :, chs], x_dram[nm][:, :, chs])

    # ---- projections, interleaved per chunk so attention can start early --
    qT = p_qk.tile([128, L], BF16, tag="qT")
    kT = p_qk.tile([128, L], BF16, tag="kT")
    v_sb = p_v.tile([LP, NT, 130], BF16, tag="v")
    nc.vector.memset(v_sb[:, :, 64:65], 1.0)
    nc.vector.memset(v_sb[:, :, 129:130], 1.0)
    TPC = max(1, CH // LP)  # key-tiles per chunk
    for ch in range(NCH):
        chs = slice(ch * CH, (ch + 1) * CH)
        for dst, w_sb, x_sb, b_sb in ((kT, wk_sb, xr_sb, bk_sb),
                                      (qT, wq_sb, xc_sb, bq_sb)):
            ps = ps_p.tile([128, CH], FP32, tag="pp")
            for kc in range(KC):
                nc.tensor.matmul(
                    ps[:],
                    lhsT=w_sb[:, kc, :],
                    rhs=x_sb[:, kc, chs],
                    start=(kc == 0),
                    stop=(kc == KC - 1),
                )
            nc.vector.tensor_scalar_add(dst[:, chs], ps[:], b_sb[:])
        # v tiles covering this chunk's key range
        for t in range(ch * TPC, (ch + 1) * TPC):
            ps = ps_p.tile([LP, 128], FP32, tag="pp")
            for kc in range(KC):
                nc.tensor.matmul(
                    ps[:],
                    lhsT=xf_sb[:, kc, t * LP:(t + 1) * LP],
                    rhs=wv_sb[:, kc, :],
                    start=(kc == 0),
                    stop=(kc == KC - 1),
                )
            # one op covering cols {0:64, 65:129} via a (2, 65)-strided view
            vt2 = v_sb[:, t, 0:130].rearrange("p (h m) -> p h m", h=2)[:, :, 0:64]
            ps2 = ps[:, :].rearrange("p (h m) -> p h m", h=2)
            bv2 = bv_bc[:LP, :].rearrange("p (h m) -> p h m", h=2)
            nc.vector.tensor_add(vt2, ps2, bv2)

    # ---- attention + output projection, chunk by chunk -------------------
    # The per-chunk tail (normalize + out-projection) is deferred and emitted
    # after the NEXT chunk's first scores, so the chunk boundary never stalls
    # the ScalarE exp stream (pending list is shared across layers).
    def make_tail(pv0, pv1, chs):
        def _tail():
            ao_t = []
            for h, pv in enumerate((pv0, pv1)):
                rc = p_sm.tile([1, CH], FP32, tag="rc")
                nc.vector.reciprocal_approx_fast(rc[:], pv[64:65, :])
                bcp = ps_p.tile([64, CH], FP32, tag="pp")
                nc.tensor.matmul(bcp[:], lhsT=ones_sb[:, 0:64].bitcast(FP32R),
                                 rhs=rc[:].bitcast(FP32R), start=True, stop=True)
                bcs = p_sm.tile([64, CH], FP32, tag="bc")
                nc.vector.tensor_copy(bcs[:], bcp[:])
                ao = p_ao.tile([64, CH], BF16, tag=f"ao{h}")
                nc.vector.tensor_mul(ao[:], pv[0:64, :], bcs[:])
                ao_t.append(ao)
            # partial out projection: outT (C, CH) = wo0^T @ ao0 + wo1^T @ ao1
            o_sb = p_out.tile([128, KC, CH], FP32, tag="osb")
            for cc in range(KC):
                ops = ps_p.tile([128, CH], FP32, tag="pp")
                nc.tensor.matmul(ops[:], lhsT=wo0_sb[:, cc * 128:(cc + 1) * 128],
                                 rhs=ao_t[0][:], start=True, stop=False)
                nc.tensor.matmul(ops[:], lhsT=wo1_sb[:, cc * 128:(cc + 1) * 128],
                                 rhs=ao_t[1][:], start=False, stop=True)
                nc.vector.tensor_copy(o_sb[:, cc, :], ops[:])
            nc.sync.dma_start(
                io[f"out{i}"][:].rearrange("(kc p) l -> p kc l", p=128)[:, :, chs],
                o_sb[:],
            )
        return _tail

    pending = pools["pending"]

    def make_chunk(ch):
        def _chunk():
            chs = slice(ch * CH, (ch + 1) * CH)
            pv0 = ps_pv.tile([65, CH], FP32, tag="pv")
            pv1 = ps_pv.tile([65, CH], FP32, tag="pv")
            prs = []
            # software-pipelined: scores(t) ... pv(t-1) ... exp(t)
            for t in range(NT):
                ts = slice(t * LP, (t + 1) * LP)
                sp = ps_s.tile([LP, 2 * CH], FP32, tag="sp")
                nc.tensor.matmul(
                    sp[:, 0:CH], lhsT=kT[0:64, ts], rhs=qT[0:64, chs],
                    start=True, stop=True,
                )
                nc.tensor.matmul(
                    sp[:, CH:2 * CH], lhsT=kT[64:128, ts], rhs=qT[64:128, chs],
                    start=True, stop=True,
                )
                if t == 1 and pending:
                    pending.pop(0)()
                if t >= 1:
                    pr_p = prs[t - 1]
                    nc.tensor.matmul(
                        pv0[:], lhsT=v_sb[:, t - 1, 0:65], rhs=pr_p[:, 0:CH],
                        start=(t - 1 == 0), stop=(t - 1 == NT - 1),
                    )
                    nc.tensor.matmul(
                        pv1[:], lhsT=v_sb[:, t - 1, 65:130], rhs=pr_p[:, CH:2 * CH],
                        start=(t - 1 == 0), stop=(t - 1 == NT - 1),
                    )
                pr = p_pr.tile([LP, 2 * CH], BF16, tag="pr")
                nc.scalar.activation(pr[:], sp[:], mybir.ActivationFunctionType.Exp,
                                     scale=float(DH) ** -0.5)
                prs.append(pr)
            t = NT - 1
            nc.tensor.matmul(
                pv0[:], lhsT=v_sb[:, t, 0:65], rhs=prs[t][:, 0:CH],
                start=(t == 0), stop=True,
            )
            nc.tensor.matmul(
                pv1[:], lhsT=v_sb[:, t, 65:130], rhs=prs[t][:, CH:2 * CH],
                start=(t == 0), stop=True,
            )
            if NT == 1 and pending:
                pending.pop(0)()
            pending.append(make_tail(pv0, pv1, chs))
        return _chunk

    return [make_chunk(ch) for ch in range(NCH)]


def _build():
    nc = bass.Bass()
    io = {}
    for i in LAYERS:
        C, R = DIMS[i], RES[i]
        L = R * R
        for nm in ("xc", "xr", "xf"):
            io[f"{nm}{i}"] = nc.declare_dram_parameter(f"{nm}{i}", [C, L], BF16, isOutput=False)
        for nm in ("wq", "wk", "wv"):
            io[f"{nm}{i}"] = nc.declare_dram_parameter(f"{nm}{i}", [C, 128], BF16, isOutput=False)
        io[f"wo{i}"] = nc.declare_dram_parameter(f"wo{i}", [128, C], BF16, isOutput=False)
        io[f"bq{i}"] = nc.declare_dram_parameter(f"bq{i}", [128, 1], FP32, isOutput=False)
        io[f"bk{i}"] = nc.declare_dram_parameter(f"bk{i}", [128, 1], FP32, isOutput=False)
        io[f"bv{i}"] = nc.declare_dram_parameter(f"bv{i}", [1, 128], FP32, isOutput=False)
        io[f"out{i}"] = nc.declare_dram_parameter(f"out{i}", [C, L], FP32, isOutput=True)

    with ExitStack() as ctx:
        tc = ctx.enter_context(tile.TileContext(nc))
        pools = {
            "x": ctx.enter_context(tc.tile_pool(name="x", bufs=2)),
            "w": ctx.enter_context(tc.tile_pool(name="w", bufs=2)),
            "qk": ctx.enter_context(tc.tile_pool(name="qk", bufs=2)),
            "v": ctx.enter_context(tc.tile_pool(name="v", bufs=2)),
            "pr": ctx.enter_context(tc.tile_pool(name="pr", bufs=6)),
            "ao": ctx.enter_context(tc.tile_pool(name="ao", bufs=4)),
            "sm": ctx.enter_context(tc.tile_pool(name="sm", bufs=8)),
            "out": ctx.enter_context(tc.tile_pool(name="out", bufs=4)),
            "const": ctx.enter_context(tc.tile_pool(name="const", bufs=1)),
            "ps_s": ctx.enter_context(tc.tile_pool(name="ps_s", bufs=2, space="PSUM")),
            "ps_pv": ctx.enter_context(tc.tile_pool(name="ps_pv", bufs=3, space="PSUM")),
            "ps_p": ctx.enter_context(tc.tile_pool(name="ps_p", bufs=1, space="PSUM")),
        }
        pools["pending"] = []  # deferred chunk tails, shared across layers
        ones_sb = pools["const"].tile([1, 128], FP32, tag="ones")
        nc.vector.memset(ones_sb[:], 1.0)
        if set(LAYERS) == {0, 1, 2, 3}:
            # Layer 0 first (dominant).  Layers 2/3's short attention chains
            # are interleaved between layer 1's chunks so they hide under
            # layer 1's ScalarE-bound exp stream.
            c0 = _emit_layer(tc, nc, pools, io, 0, ones_sb)
            for f in c0:
                f()
            # flush layer 0's last tail here: it overlaps with the small
            # layers' prep work and frees its PSUM accumulators early
            while pools["pending"]:
                pools["pending"].pop(0)()
            c2 = _emit_layer(tc, nc, pools, io, 2, ones_sb)
            c3 = _emit_layer(tc, nc, pools, io, 3, ones_sb)
            c1 = _emit_layer(tc, nc, pools, io, 1, ones_sb)
            for f in (c2[0], c3[0], c1[0], c1[1]):
                f()
        else:
            for i in LAYERS:
                for f in _emit_layer(tc, nc, pools, io, i, ones_sb):
                    f()
        for tail in pools["pending"]:
            tail()
        pools["pending"] = []
    return nc


def _core_inputs(inputs, c):
    b, p = c // 4, c % 4
    sl = slice(p * 128, (p + 1) * 128)
    m = {}
    for i in LAYERS:
        C, R = DIMS[i], RES[i]
        L = R * R
        for nm, src in (("xc", "ml_c"), ("xr", "ml_r"), ("xf", "fl_r")):
            x = np.asarray(inputs[f"{src}{i}"][b], np.float32).reshape(C, L)
            m[f"{nm}{i}"] = np.ascontiguousarray(x).astype(NP_BF16)
        for nm in ("wq", "wk", "wv"):
            w = np.asarray(inputs[f"{nm}{i}"], np.float32)[:, sl]
            m[f"{nm}{i}"] = np.ascontiguousarray(w).astype(NP_BF16)
        wo = np.asarray(inputs[f"wo{i}"], np.float32)[sl, :]
        m[f"wo{i}"] = np.ascontiguousarray(wo).astype(NP_BF16)
        m[f"bq{i}"] = np.ascontiguousarray(
            np.asarray(inputs[f"bq{i}"], np.float32)[sl].reshape(128, 1))
        m[f"bk{i}"] = np.ascontiguousarray(
            np.asarray(inputs[f"bk{i}"], np.float32)[sl].reshape(128, 1))
        m[f"bv{i}"] = np.ascontiguousarray(
            np.asarray(inputs[f"bv{i}"], np.float32)[sl].reshape(1, 128))
    return m


def kernel(**inputs):
    global _NC, LAST_RESULTS
    if _NC is None:
        _NC = _build()
    in_maps = [_core_inputs(inputs, c) for c in range(N_CORES)]
    res = run_bass_kernel_spmd(
        _NC,
        in_maps,
        core_ids=list(range(N_CORES)),
        trace=bool(int(os.environ.get("KERNEL_TRACE", "0"))),
    )
    LAST_RESULTS = res
    outs = []
    for i in range(4):
        C, R = DIMS[i], RES[i]
        L = R * R
        acc = np.zeros((B, C, L), np.float32)
        if i in LAYERS:
            for c in range(N_CORES):
                acc[c // 4] += res.results[c][f"out{i}"]
            acc += np.asarray(inputs[f"bo{i}"], np.float32)[None, :, None]
        outs.append(acc.reshape(B, C, R, R))
    return tuple(outs)


# revision 16
# speedup vs baseline: 1.0537x; 1.0537x over previous
"""Trainium2 Bass/Tile kernel for nn_CrossAttentionModule (4 cross-attention layers).

Sharding (8 cores): core c handles batch b = c//4 and head-pair p = c%4
(heads 2p, 2p+1)  -- data-parallel over batch, tensor-parallel over heads.
Each core computes q/k/v projections for its 128-wide slice of INNER, the
attention for its two heads, and a PARTIAL output projection (its heads'
contribution).  The host sums the 4 partials per batch and adds the output
bias.

Per-layer on-device dataflow (everything in "transposed" layout, so no
transposes are ever needed):
  xT (C, L) comes straight from HBM (ml_* tensors are (B, C, H, W)).
  qT (128, L)  = wq_slice^T @ xcT          (K=C, accumulated over C/128 chunks)
  kT (128, L)  = wk_slice^T @ xrT
  v  (L, 128)  = xfT^T @ wv_slice          (keys on partitions)
  scoresT tile (Lk, Lq) = kT_head^T-free @ qT_head  (K=64; the two heads live
     on disjoint partition ranges 0:64 / 64:128 so their score matmuls run
     concurrently in disjoint PE row-groups)
  probsT = exp(scoresT / 8)                (ScalarE, no max subtraction:
                                            scores are ~N(0,1) so exp is safe)
  attn_outT (65, Lq) += [v_head | ones]^T @ probsT   (ones column produces the
                                            softmax denominator in row 64)
  normalize: recip = 1/denom (DVE approx), broadcast across partitions via a
     K=1 PE outer-product, multiply.
  partial outT (C, Lq) = wo_h0^T @ attn0 + wo_h1^T @ attn1   (two K=64 matmuls)

Matmuls run in bf16 (fp32 matmul is 4x slower on TRN2 PE); accumulation is
fp32 in PSUM.  Softmax numerator/denominator use the same bf16 probs, so
rounding largely cancels in the ratio.
"""

import os
from contextlib import ExitStack

import ml_dtypes
import numpy as np

import concourse.bass as bass
import concourse.mybir as mybir
import concourse.tile as tile
from concourse.bass_utils import run_bass_kernel_spmd

BF16 = mybir.dt.bfloat16
FP32 = mybir.dt.float32
FP32R = mybir.dt.float32r
NP_BF16 = ml_dtypes.bfloat16

DIMS = [128, 256, 512, 512]
RES = [64, 32, 16, 8]
B = 2
DH = 64
N_CORES = 8

LAYERS = tuple(
    int(x) for x in os.environ.get("KERNEL_LAYERS", "0,1,2,3").split(",") if x != ""
)

_NC = None
LAST_RESULTS = None


def _emit_layer(tc, nc, pools, io, i, ones_sb):
    C, R = DIMS[i], RES[i]
    L = R * R
    KC = C // 128          # contraction chunks for projections
    CH = min(512, L)       # query-chunk width (free dim of PSUM tiles)
    NCH = L // CH
    LP = min(L, 128)       # key-tile partition size
    NT = L // LP           # number of key tiles

    p_x, p_w, p_qk, p_v, p_pr, p_ao, p_sm, p_out = (
        pools["x"], pools["w"], pools["qk"], pools["v"], pools["pr"],
        pools["ao"], pools["sm"], pools["out"],
    )
    ps_s, ps_pv, ps_p = pools["ps_s"], pools["ps_pv"], pools["ps_p"]

    # ---- load weights / biases ------------------------------------------
    wq_sb = p_w.tile([128, KC, 128], BF16, tag=f"wq{i}", bufs=1)
    wk_sb = p_w.tile([128, KC, 128], BF16, tag=f"wk{i}", bufs=1)
    wv_sb = p_w.tile([128, KC, 128], BF16, tag=f"wv{i}", bufs=1)
    for w_sb, nm in ((wq_sb, "wq"), (wk_sb, "wk"), (wv_sb, "wv")):
        nc.sync.dma_start(
            w_sb[:], io[f"{nm}{i}"][:].rearrange("(kc p) m -> p kc m", p=128)
        )
    wo0_sb = p_w.tile([64, C], BF16, tag=f"wo0{i}", bufs=1)   # rows 0:64  of wo slice (head 0 dims)
    wo1_sb = p_w.tile([64, C], BF16, tag=f"wo1{i}", bufs=1)   # rows 64:128 (head 1 dims)
    nc.sync.dma_start(wo0_sb[:], io[f"wo{i}"][0:64, :])
    nc.sync.dma_start(wo1_sb[:], io[f"wo{i}"][64:128, :])
    bq_sb = p_w.tile([128, 1], FP32, tag=f"bq{i}", bufs=1)
    bk_sb = p_w.tile([128, 1], FP32, tag=f"bk{i}", bufs=1)
    nc.sync.dma_start(bq_sb[:], io[f"bq{i}"][:])
    nc.sync.dma_start(bk_sb[:], io[f"bk{i}"][:])
    bv_sb = p_w.tile([1, 128], FP32, tag=f"bv{i}", bufs=1)
    nc.sync.dma_start(bv_sb[:], io[f"bv{i}"][:])
    # broadcast bv across partitions with a K=1 outer product: bvb[m, n] = bv[n]
    # (fp32r bitcast: full-rate streaming vs 4 cycles/row for plain fp32)
    bvb_ps = ps_p.tile([128, 128], FP32, tag="pp")
    nc.tensor.matmul(bvb_ps[:], lhsT=ones_sb[:, 0:128].bitcast(FP32R),
                     rhs=bv_sb[:].bitcast(FP32R), start=True, stop=True)
    bv_bc = p_w.tile([128, 128], FP32, tag=f"bvbc{i}", bufs=1)
    nc.vector.tensor_copy(bv_bc[:], bvb_ps[:])

    # ---- load activations (already bf16, already transposed as (C, L)) ---
    # Chunked DMAs so the first projections / scores never wait on the whole
    # tensor transfer.
    xc_sb = p_x.tile([128, KC, L], BF16, tag=f"xc{i}", bufs=1)
    xr_sb = p_x.tile([128, KC, L], BF16, tag=f"xr{i}", bufs=1)
    xf_sb = p_x.tile([128, KC, L], BF16, tag=f"xf{i}", bufs=1)
    x_dram = {
        nm: io[f"{nm}{i}"][:].rearrange("(kc p) l -> p kc l", p=128)
        for nm in ("xc", "xr", "xf")
    }
    for ch in range(NCH):
        chs = slice(ch * CH, (ch + 1) * CH)
        for x_sb, nm in ((xc_sb, "xc"), (xr_sb, "xr"), (xf_sb, "xf")):
            nc.sync.dma_start(x_sb[:, :, chs], x_dram[nm][:, :, chs])

    # ---- projections, interleaved per chunk so attention can start early --
    qT = p_qk.tile([128, L], BF16, tag=f"qT{i}", bufs=1)
    kT = p_qk.tile([128, L], BF16, tag=f"kT{i}", bufs=1)
    v_sb = p_v.tile([LP, NT, 130], BF16, tag=f"v{i}", bufs=1)
    nc.vector.memset(v_sb[:, :, 64:65], 1.0)
    nc.vector.memset(v_sb[:, :, 129:130], 1.0)
    TPC = max(1, CH // LP)  # key-tiles per chunk
    for ch in range(NCH):
        chs = slice(ch * CH, (ch + 1) * CH)
        for dst, w_sb, x_sb, b_sb in ((kT, wk_sb, xr_sb, bk_sb),
                                      (qT, wq_sb, xc_sb, bq_sb)):
            ps = ps_p.tile([128, CH], FP32, tag="pp")
            for kc in range(KC):
                nc.tensor.matmul(
                    ps[:],
                    lhsT=w_sb[:, kc, :],
                    rhs=x_sb[:, kc, chs],
                    start=(kc == 0),
                    stop=(kc == KC - 1),
                )
            nc.vector.tensor_scalar_add(dst[:, chs], ps[:], b_sb[:])
        # v tiles covering this chunk's key range
        for t in range(ch * TPC, (ch + 1) * TPC):
            ps = ps_p.tile([LP, 128], FP32, tag="pp")
            for kc in range(KC):
                nc.tensor.matmul(
                    ps[:],
                    lhsT=xf_sb[:, kc, t * LP:(t + 1) * LP],
                    rhs=wv_sb[:, kc, :],
                    start=(kc == 0),
                    stop=(kc == KC - 1),
                )
            # one op covering cols {0:64, 65:129} via a (2, 65)-strided view
            vt2 = v_sb[:, t, 0:130].rearrange("p (h m) -> p h m", h=2)[:, :, 0:64]
            ps2 = ps[:, :].rearrange("p (h m) -> p h m", h=2)
            bv2 = bv_bc[:LP, :].rearrange("p (h m) -> p h m", h=2)
            nc.vector.tensor_add(vt2, ps2, bv2)

    # ---- attention + output projection, chunk by chunk -------------------
    # The per-chunk tail (normalize + out-projection) is deferred and emitted
    # after the NEXT chunk's first scores, so the chunk boundary never stalls
    # the ScalarE exp stream (pending list is shared across layers).
    def make_tail(pv0, pv1, chs):
        def _tail():
            ao_t = []
            for h, pv in enumerate((pv0, pv1)):
                rc = p_sm.tile([1, CH], FP32, tag="rc")
                nc.vector.reciprocal_approx_fast(rc[:], pv[64:65, :])
                bcp = ps_p.tile([64, CH], FP32, tag="pp")
                nc.tensor.matmul(bcp[:], lhsT=ones_sb[:, 0:64].bitcast(FP32R),
                                 rhs=rc[:].bitcast(FP32R), start=True, stop=True)
                bcs = p_sm.tile([64, CH], FP32, tag="bc")
                nc.vector.tensor_copy(bcs[:], bcp[:])
                ao = p_ao.tile([64, CH], BF16, tag=f"ao{h}")
                nc.vector.tensor_mul(ao[:], pv[0:64, :], bcs[:])
                ao_t.append(ao)
            # partial out projection: outT (C, CH) = wo0^T @ ao0 + wo1^T @ ao1
            o_sb = p_out.tile([128, KC, CH], FP32, tag="osb")
            for cc in range(KC):
                ops = ps_p.tile([128, CH], FP32, tag="pp")
                nc.tensor.matmul(ops[:], lhsT=wo0_sb[:, cc * 128:(cc + 1) * 128],
                                 rhs=ao_t[0][:], start=True, stop=False)
                nc.tensor.matmul(ops[:], lhsT=wo1_sb[:, cc * 128:(cc + 1) * 128],
                                 rhs=ao_t[1][:], start=False, stop=True)
                nc.vector.tensor_copy(o_sb[:, cc, :], ops[:])
            nc.sync.dma_start(
                io[f"out{i}"][:].rearrange("(kc p) l -> p kc l", p=128)[:, :, chs],
                o_sb[:],
            )
        return _tail

    pending = pools["pending"]

    def make_chunk(ch):
        def _chunk():
            chs = slice(ch * CH, (ch + 1) * CH)
            pv0 = ps_pv.tile([65, CH], FP32, tag="pv")
            pv1 = ps_pv.tile([65, CH], FP32, tag="pv")
            prs = []
            # software-pipelined: scores(t) ... pv(t-1) ... exp(t)
            for t in range(NT):
                ts = slice(t * LP, (t + 1) * LP)
                sp = ps_s.tile([LP, 2 * CH], FP32, tag="sp")
                nc.tensor.matmul(
                    sp[:, 0:CH], lhsT=kT[0:64, ts], rhs=qT[0:64, chs],
                    start=True, stop=True,
                )
                nc.tensor.matmul(
                    sp[:, CH:2 * CH], lhsT=kT[64:128, ts], rhs=qT[64:128, chs],
                    start=True, stop=True,
                )
                if t == 1 and pending:
                    pending.pop(0)()
                if t >= 1:
                    pr_p = prs[t - 1]
                    nc.tensor.matmul(
                        pv0[:], lhsT=v_sb[:, t - 1, 0:65], rhs=pr_p[:, 0:CH],
                        start=(t - 1 == 0), stop=(t - 1 == NT - 1),
                    )
                    nc.tensor.matmul(
                        pv1[:], lhsT=v_sb[:, t - 1, 65:130], rhs=pr_p[:, CH:2 * CH],
                        start=(t - 1 == 0), stop=(t - 1 == NT - 1),
                    )
                pr = p_pr.tile([LP, 2 * CH], BF16, tag="pr")
                nc.scalar.activation(pr[:], sp[:], mybir.ActivationFunctionType.Exp,
                                     scale=float(DH) ** -0.5)
                prs.append(pr)
            t = NT - 1
            nc.tensor.matmul(
                pv0[:], lhsT=v_sb[:, t, 0:65], rhs=prs[t][:, 0:CH],
                start=(t == 0), stop=True,
            )
            nc.tensor.matmul(
                pv1[:], lhsT=v_sb[:, t, 65:130], rhs=prs[t][:, CH:2 * CH],
                start=(t == 0), stop=True,
            )
            if NT == 1 and pending:
                pending.pop(0)()
            pending.append(make_tail(pv0, pv1, chs))
        return _chunk

    return [make_chunk(ch) for ch in range(NCH)]


def _build():
    nc = bass.Bass()
    io = {}
    for i in LAYERS:
        C, R = DIMS[i], RES[i]
        L = R * R
        for nm in ("xc", "xr", "xf"):
            io[f"{nm}{i}"] = nc.declare_dram_parameter(f"{nm}{i}", [C, L], BF16, isOutput=False)
        for nm in ("wq", "wk", "wv"):
            io[f"{nm}{i}"] = nc.declare_dram_parameter(f"{nm}{i}", [C, 128], BF16, isOutput=False)
        io[f"wo{i}"] = nc.declare_dram_parameter(f"wo{i}", [128, C], BF16, isOutput=False)
        io[f"bq{i}"] = nc.declare_dram_parameter(f"bq{i}", [128, 1], FP32, isOutput=False)
        io[f"bk{i}"] = nc.declare_dram_parameter(f"bk{i}", [128, 1], FP32, isOutput=False)
        io[f"bv{i}"] = nc.declare_dram_parameter(f"bv{i}", [1, 128], FP32, isOutput=False)
        io[f"out{i}"] = nc.declare_dram_parameter(f"out{i}", [C, L], FP32, isOutput=True)

    with ExitStack() as ctx:
        tc = ctx.enter_context(tile.TileContext(nc))
        pools = {
            "x": ctx.enter_context(tc.tile_pool(name="x", bufs=2)),
            "w": ctx.enter_context(tc.tile_pool(name="w", bufs=2)),
            "qk": ctx.enter_context(tc.tile_pool(name="qk", bufs=2)),
            "v": ctx.enter_context(tc.tile_pool(name="v", bufs=2)),
            "pr": ctx.enter_context(tc.tile_pool(name="pr", bufs=6)),
            "ao": ctx.enter_context(tc.tile_pool(name="ao", bufs=4)),
            "sm": ctx.enter_context(tc.tile_pool(name="sm", bufs=8)),
            "out": ctx.enter_context(tc.tile_pool(name="out", bufs=4)),
            "const": ctx.enter_context(tc.tile_pool(name="const", bufs=1)),
            "ps_s": ctx.enter_context(tc.tile_pool(name="ps_s", bufs=2, space="PSUM")),
            "ps_pv": ctx.enter_context(tc.tile_pool(name="ps_pv", bufs=3, space="PSUM")),
            "ps_p": ctx.enter_context(tc.tile_pool(name="ps_p", bufs=1, space="PSUM")),
        }
        pools["pending"] = []  # deferred chunk tails, shared across layers
        ones_sb = pools["const"].tile([1, 128], FP32, tag="ones")
        nc.vector.memset(ones_sb[:], 1.0)
        if set(LAYERS) == {0, 1, 2, 3}:
            # Layer 0 first (dominant).  Layers 2/3's short attention chains
            # are interleaved between layer 1's chunks so they hide under
            # layer 1's ScalarE-bound exp stream.
            c0 = _emit_layer(tc, nc, pools, io, 0, ones_sb)
            for f in c0:
                f()
            c2 = _emit_layer(tc, nc, pools, io, 2, ones_sb)
            c3 = _emit_layer(tc, nc, pools, io, 3, ones_sb)
            c1 = _emit_layer(tc, nc, pools, io, 1, ones_sb)
            for f in (c2[0], c3[0], c1[0], c1[1]):
                f()
        else:
            for i in LAYERS:
                for f in _emit_layer(tc, nc, pools, io, i, ones_sb):
                    f()
        for tail in pools["pending"]:
            tail()
        pools["pending"] = []
    return nc


def _core_inputs(inputs, c):
    b, p = c // 4, c % 4
    sl = slice(p * 128, (p + 1) * 128)
    m = {}
    for i in LAYERS:
        C, R = DIMS[i], RES[i]
        L = R * R
        for nm, src in (("xc", "ml_c"), ("xr", "ml_r"), ("xf", "fl_r")):
            x = np.asarray(inputs[f"{src}{i}"][b], np.float32).reshape(C, L)
            m[f"{nm}{i}"] = np.ascontiguousarray(x).astype(NP_BF16)
        for nm in ("wq", "wk", "wv"):
            w = np.asarray(inputs[f"{nm}{i}"], np.float32)[:, sl]
            m[f"{nm}{i}"] = np.ascontiguousarray(w).astype(NP_BF16)
        wo = np.asarray(inputs[f"wo{i}"], np.float32)[sl, :]
        m[f"wo{i}"] = np.ascontiguousarray(wo).astype(NP_BF16)
        m[f"bq{i}"] = np.ascontiguousarray(
            np.asarray(inputs[f"bq{i}"], np.float32)[sl].reshape(128, 1))
        m[f"bk{i}"] = np.ascontiguousarray(
            np.asarray(inputs[f"bk{i}"], np.float32)[sl].reshape(128, 1))
        m[f"bv{i}"] = np.ascontiguousarray(
            np.asarray(inputs[f"bv{i}"], np.float32)[sl].reshape(1, 128))
    return m


def kernel(**inputs):
    global _NC, LAST_RESULTS
    if _NC is None:
        _NC = _build()
    in_maps = [_core_inputs(inputs, c) for c in range(N_CORES)]
    res = run_bass_kernel_spmd(
        _NC,
        in_maps,
        core_ids=list(range(N_CORES)),
        trace=bool(int(os.environ.get("KERNEL_TRACE", "0"))),
    )
    LAST_RESULTS = res
    outs = []
    for i in range(4):
        C, R = DIMS[i], RES[i]
        L = R * R
        acc = np.zeros((B, C, L), np.float32)
        if i in LAYERS:
            for c in range(N_CORES):
                acc[c // 4] += res.results[c][f"out{i}"]
            acc += np.asarray(inputs[f"bo{i}"], np.float32)[None, :, None]
        outs.append(acc.reshape(B, C, R, R))
    return tuple(outs)


# revision 17
# speedup vs baseline: 1.0547x; 1.0010x over previous
"""Trainium2 Bass/Tile kernel for nn_CrossAttentionModule (4 cross-attention layers).

Sharding (8 cores): core c handles batch b = c//4 and head-pair p = c%4
(heads 2p, 2p+1)  -- data-parallel over batch, tensor-parallel over heads.
Each core computes q/k/v projections for its 128-wide slice of INNER, the
attention for its two heads, and a PARTIAL output projection (its heads'
contribution).  The host sums the 4 partials per batch and adds the output
bias.

Per-layer on-device dataflow (everything in "transposed" layout, so no
transposes are ever needed):
  xT (C, L) comes straight from HBM (ml_* tensors are (B, C, H, W)).
  qT (128, L)  = wq_slice^T @ xcT          (K=C, accumulated over C/128 chunks)
  kT (128, L)  = wk_slice^T @ xrT
  v  (L, 128)  = xfT^T @ wv_slice          (keys on partitions)
  scoresT tile (Lk, Lq) = kT_head^T-free @ qT_head  (K=64; the two heads live
     on disjoint partition ranges 0:64 / 64:128 so their score matmuls run
     concurrently in disjoint PE row-groups)
  probsT = exp(scoresT / 8)                (ScalarE, no max subtraction:
                                            scores are ~N(0,1) so exp is safe)
  attn_outT (65, Lq) += [v_head | ones]^T @ probsT   (ones column produces the
                                            softmax denominator in row 64)
  normalize: recip = 1/denom (DVE approx), broadcast across partitions via a
     K=1 PE outer-product, multiply.
  partial outT (C, Lq) = wo_h0^T @ attn0 + wo_h1^T @ attn1   (two K=64 matmuls)

Matmuls run in bf16 (fp32 matmul is 4x slower on TRN2 PE); accumulation is
fp32 in PSUM.  Softmax numerator/denominator use the same bf16 probs, so
rounding largely cancels in the ratio.
"""

import os
from contextlib import ExitStack

import ml_dtypes
import numpy as np

import concourse.bass as bass
import concourse.mybir as mybir
import concourse.tile as tile
from concourse.bass_utils import run_bass_kernel_spmd

BF16 = mybir.dt.bfloat16
FP32 = mybir.dt.float32
FP32R = mybir.dt.float32r
NP_BF16 = ml_dtypes.bfloat16

DIMS = [128, 256, 512, 512]
RES = [64, 32, 16, 8]
B = 2
DH = 64
N_CORES = 8

LAYERS = tuple(
    int(x) for x in os.environ.get("KERNEL_LAYERS", "0,1,2,3").split(",") if x != ""
)

_NC = None
LAST_RESULTS = None


def _emit_layer(tc, nc, pools, io, i, ones_sb):
    C, R = DIMS[i], RES[i]
    L = R * R
    KC = C // 128          # contraction chunks for projections
    CH = min(512, L)       # query-chunk width (free dim of PSUM tiles)
    NCH = L // CH
    LP = min(L, 128)       # key-tile partition size
    NT = L // LP           # number of key tiles

    p_x, p_w, p_qk, p_v, p_pr, p_ao, p_sm, p_out = (
        pools["x"], pools["w"], pools["qk"], pools["v"], pools["pr"],
        pools["ao"], pools["sm"], pools["out"],
    )
    ps_s, ps_pv, ps_p = pools["ps_s"], pools["ps_pv"], pools["ps_p"]

    # ---- load weights / biases ------------------------------------------
    wq_sb = p_w.tile([128, KC, 128], BF16, tag=f"wq{i}", bufs=1)
    wk_sb = p_w.tile([128, KC, 128], BF16, tag=f"wk{i}", bufs=1)
    wv_sb = p_w.tile([128, KC, 128], BF16, tag=f"wv{i}", bufs=1)
    for w_sb, nm in ((wq_sb, "wq"), (wk_sb, "wk"), (wv_sb, "wv")):
        nc.sync.dma_start(
            w_sb[:], io[f"{nm}{i}"][:].rearrange("(kc p) m -> p kc m", p=128)
        )
    wo0_sb = p_w.tile([64, C], BF16, tag=f"wo0{i}", bufs=1)   # rows 0:64  of wo slice (head 0 dims)
    wo1_sb = p_w.tile([64, C], BF16, tag=f"wo1{i}", bufs=1)   # rows 64:128 (head 1 dims)
    nc.sync.dma_start(wo0_sb[:], io[f"wo{i}"][0:64, :])
    nc.sync.dma_start(wo1_sb[:], io[f"wo{i}"][64:128, :])
    bq_sb = p_w.tile([128, 1], FP32, tag=f"bq{i}", bufs=1)
    bk_sb = p_w.tile([128, 1], FP32, tag=f"bk{i}", bufs=1)
    nc.sync.dma_start(bq_sb[:], io[f"bq{i}"][:])
    nc.sync.dma_start(bk_sb[:], io[f"bk{i}"][:])
    bv_sb = p_w.tile([1, 128], FP32, tag=f"bv{i}", bufs=1)
    nc.sync.dma_start(bv_sb[:], io[f"bv{i}"][:])
    # broadcast bv across partitions with a K=1 outer product: bvb[m, n] = bv[n]
    # (fp32r bitcast: full-rate streaming vs 4 cycles/row for plain fp32)
    bvb_ps = ps_p.tile([128, 128], FP32, tag="pp")
    nc.tensor.matmul(bvb_ps[:], lhsT=ones_sb[:, 0:128].bitcast(FP32R),
                     rhs=bv_sb[:].bitcast(FP32R), start=True, stop=True)
    bv_bc = p_w.tile([128, 128], FP32, tag=f"bvbc{i}", bufs=1)
    nc.vector.tensor_copy(bv_bc[:], bvb_ps[:])

    # ---- load activations (already bf16, already transposed as (C, L)) ---
    # Chunked DMAs so the first projections / scores never wait on the whole
    # tensor transfer.
    xc_sb = p_x.tile([128, KC, L], BF16, tag=f"xc{i}", bufs=1)
    xr_sb = p_x.tile([128, KC, L], BF16, tag=f"xr{i}", bufs=1)
    xf_sb = p_x.tile([128, KC, L], BF16, tag=f"xf{i}", bufs=1)
    x_dram = {
        nm: io[f"{nm}{i}"][:].rearrange("(kc p) l -> p kc l", p=128)
        for nm in ("xc", "xr", "xf")
    }
    for ch in range(NCH):
        chs = slice(ch * CH, (ch + 1) * CH)
        for x_sb, nm in ((xc_sb, "xc"), (xr_sb, "xr"), (xf_sb, "xf")):
            nc.sync.dma_start(x_sb[:, :, chs], x_dram[nm][:, :, chs])

    # ---- projections, interleaved per chunk so attention can start early --
    qT = p_qk.tile([128, L], BF16, tag=f"qT{i}", bufs=1)
    kT = p_qk.tile([128, L], BF16, tag=f"kT{i}", bufs=1)
    v_sb = p_v.tile([LP, NT, 130], BF16, tag=f"v{i}", bufs=1)
    nc.vector.memset(v_sb[:, :, 64:65], 1.0)
    nc.vector.memset(v_sb[:, :, 129:130], 1.0)
    TPC = max(1, CH // LP)  # key-tiles per chunk
    for ch in range(NCH):
        chs = slice(ch * CH, (ch + 1) * CH)
        for dst, w_sb, x_sb, b_sb in ((kT, wk_sb, xr_sb, bk_sb),
                                      (qT, wq_sb, xc_sb, bq_sb)):
            ps = ps_p.tile([128, CH], FP32, tag="pp")
            for kc in range(KC):
                nc.tensor.matmul(
                    ps[:],
                    lhsT=w_sb[:, kc, :],
                    rhs=x_sb[:, kc, chs],
                    start=(kc == 0),
                    stop=(kc == KC - 1),
                )
            nc.vector.tensor_scalar_add(dst[:, chs], ps[:], b_sb[:])
        # v tiles covering this chunk's key range
        for t in range(ch * TPC, (ch + 1) * TPC):
            ps = ps_p.tile([LP, 128], FP32, tag="pp")
            for kc in range(KC):
                nc.tensor.matmul(
                    ps[:],
                    lhsT=xf_sb[:, kc, t * LP:(t + 1) * LP],
                    rhs=wv_sb[:, kc, :],
                    start=(kc == 0),
                    stop=(kc == KC - 1),
                )
            # one op covering cols {0:64, 65:129} via a (2, 65)-strided view
            vt2 = v_sb[:, t, 0:130].rearrange("p (h m) -> p h m", h=2)[:, :, 0:64]
            ps2 = ps[:, :].rearrange("p (h m) -> p h m", h=2)
            bv2 = bv_bc[:LP, :].rearrange("p (h m) -> p h m", h=2)
            nc.vector.tensor_add(vt2, ps2, bv2)

    # ---- attention + output projection, chunk by chunk -------------------
    # The per-chunk tail (normalize + out-projection) is deferred and emitted
    # after the NEXT chunk's first scores, so the chunk boundary never stalls
    # the ScalarE exp stream (pending list is shared across layers).
    def make_tail(pv0, pv1, chs):
        def _tail():
            ao_t = []
            for h, pv in enumerate((pv0, pv1)):
                rc = p_sm.tile([1, CH], FP32, tag="rc")
                nc.vector.reciprocal_approx_fast(rc[:], pv[64:65, :])
                bcp = ps_p.tile([64, CH], FP32, tag="pp")
                nc.tensor.matmul(bcp[:], lhsT=ones_sb[:, 0:64].bitcast(FP32R),
                                 rhs=rc[:].bitcast(FP32R), start=True, stop=True)
                bcs = p_sm.tile([64, CH], FP32, tag="bc")
                nc.vector.tensor_copy(bcs[:], bcp[:])
                ao = p_ao.tile([64, CH], BF16, tag=f"ao{h}")
                nc.vector.tensor_mul(ao[:], pv[0:64, :], bcs[:])
                ao_t.append(ao)
            # partial out projection: outT (C, CH) = wo0^T @ ao0 + wo1^T @ ao1
            o_sb = p_out.tile([128, KC, CH], FP32, tag="osb")
            for cc in range(KC):
                ops = ps_p.tile([128, CH], FP32, tag="pp")
                nc.tensor.matmul(ops[:], lhsT=wo0_sb[:, cc * 128:(cc + 1) * 128],
                                 rhs=ao_t[0][:], start=True, stop=False)
                nc.tensor.matmul(ops[:], lhsT=wo1_sb[:, cc * 128:(cc + 1) * 128],
                                 rhs=ao_t[1][:], start=False, stop=True)
                nc.vector.tensor_copy(o_sb[:, cc, :], ops[:])
            nc.sync.dma_start(
                io[f"out{i}"][:].rearrange("(kc p) l -> p kc l", p=128)[:, :, chs],
                o_sb[:],
            )
        return _tail

    pending = pools["pending"]

    def make_chunk(ch):
        def _chunk():
            chs = slice(ch * CH, (ch + 1) * CH)
            pv0 = ps_pv.tile([65, CH], FP32, tag="pv")
            pv1 = ps_pv.tile([65, CH], FP32, tag="pv")
            prs = []
            # software-pipelined: scores(t) ... pv(t-1) ... exp(t)
            for t in range(NT):
                ts = slice(t * LP, (t + 1) * LP)
                sp = ps_s.tile([LP, 2 * CH], FP32, tag="sp")
                nc.tensor.matmul(
                    sp[:, 0:CH], lhsT=kT[0:64, ts], rhs=qT[0:64, chs],
                    start=True, stop=True,
                )
                nc.tensor.matmul(
                    sp[:, CH:2 * CH], lhsT=kT[64:128, ts], rhs=qT[64:128, chs],
                    start=True, stop=True,
                )
                if t == 1 and pending:
                    pending.pop(0)()
                if t >= 1:
                    pr_p = prs[t - 1]
                    nc.tensor.matmul(
                        pv0[:], lhsT=v_sb[:, t - 1, 0:65], rhs=pr_p[:, 0:CH],
                        start=(t - 1 == 0), stop=(t - 1 == NT - 1),
                    )
                    nc.tensor.matmul(
                        pv1[:], lhsT=v_sb[:, t - 1, 65:130], rhs=pr_p[:, CH:2 * CH],
                        start=(t - 1 == 0), stop=(t - 1 == NT - 1),
                    )
                pr = p_pr.tile([LP, 2 * CH], BF16, tag="pr")
                nc.scalar.activation(pr[:], sp[:], mybir.ActivationFunctionType.Exp,
                                     scale=float(DH) ** -0.5)
                prs.append(pr)
            t = NT - 1
            nc.tensor.matmul(
                pv0[:], lhsT=v_sb[:, t, 0:65], rhs=prs[t][:, 0:CH],
                start=(t == 0), stop=True,
            )
            nc.tensor.matmul(
                pv1[:], lhsT=v_sb[:, t, 65:130], rhs=prs[t][:, CH:2 * CH],
                start=(t == 0), stop=True,
            )
            if NT == 1 and pending:
                pending.pop(0)()
            pending.append(make_tail(pv0, pv1, chs))
        return _chunk

    return [make_chunk(ch) for ch in range(NCH)]


def _build():
    nc = bass.Bass()
    io = {}
    for i in LAYERS:
        C, R = DIMS[i], RES[i]
        L = R * R
        for nm in ("xc", "xr", "xf"):
            io[f"{nm}{i}"] = nc.declare_dram_parameter(f"{nm}{i}", [C, L], BF16, isOutput=False)
        for nm in ("wq", "wk", "wv"):
            io[f"{nm}{i}"] = nc.declare_dram_parameter(f"{nm}{i}", [C, 128], BF16, isOutput=False)
        io[f"wo{i}"] = nc.declare_dram_parameter(f"wo{i}", [128, C], BF16, isOutput=False)
        io[f"bq{i}"] = nc.declare_dram_parameter(f"bq{i}", [128, 1], FP32, isOutput=False)
        io[f"bk{i}"] = nc.declare_dram_parameter(f"bk{i}", [128, 1], FP32, isOutput=False)
        io[f"bv{i}"] = nc.declare_dram_parameter(f"bv{i}", [1, 128], FP32, isOutput=False)
        io[f"out{i}"] = nc.declare_dram_parameter(f"out{i}", [C, L], FP32, isOutput=True)

    with ExitStack() as ctx:
        tc = ctx.enter_context(tile.TileContext(nc))
        pools = {
            "x": ctx.enter_context(tc.tile_pool(name="x", bufs=2)),
            "w": ctx.enter_context(tc.tile_pool(name="w", bufs=2)),
            "qk": ctx.enter_context(tc.tile_pool(name="qk", bufs=2)),
            "v": ctx.enter_context(tc.tile_pool(name="v", bufs=2)),
            "pr": ctx.enter_context(tc.tile_pool(name="pr", bufs=6)),
            "ao": ctx.enter_context(tc.tile_pool(name="ao", bufs=4)),
            "sm": ctx.enter_context(tc.tile_pool(name="sm", bufs=8)),
            "out": ctx.enter_context(tc.tile_pool(name="out", bufs=4)),
            "const": ctx.enter_context(tc.tile_pool(name="const", bufs=1)),
            "ps_s": ctx.enter_context(tc.tile_pool(name="ps_s", bufs=2, space="PSUM")),
            "ps_pv": ctx.enter_context(tc.tile_pool(name="ps_pv", bufs=3, space="PSUM")),
            "ps_p": ctx.enter_context(tc.tile_pool(name="ps_p", bufs=1, space="PSUM")),
        }
        pools["pending"] = []  # deferred chunk tails, shared across layers
        ones_sb = pools["const"].tile([1, 128], FP32, tag="ones")
        nc.vector.memset(ones_sb[:], 1.0)
        if set(LAYERS) == {0, 1, 2, 3}:
            # Layer 0 first (dominant).  Layers 2/3's short attention chains
            # are interleaved between layer 1's chunks so they hide under
            # layer 1's ScalarE-bound exp stream.
            c0 = _emit_layer(tc, nc, pools, io, 0, ones_sb)
            for f in c0:
                f()
            c2 = _emit_layer(tc, nc, pools, io, 2, ones_sb)
            c3 = _emit_layer(tc, nc, pools, io, 3, ones_sb)
            c1 = _emit_layer(tc, nc, pools, io, 1, ones_sb)
            for f in (c1[0], c2[0], c1[1], c3[0]):
                f()
        else:
            for i in LAYERS:
                for f in _emit_layer(tc, nc, pools, io, i, ones_sb):
                    f()
        for tail in pools["pending"]:
            tail()
        pools["pending"] = []
    return nc


def _core_inputs(inputs, c):
    b, p = c // 4, c % 4
    sl = slice(p * 128, (p + 1) * 128)
    m = {}
    for i in LAYERS:
        C, R = DIMS[i], RES[i]
        L = R * R
        for nm, src in (("xc", "ml_c"), ("xr", "ml_r"), ("xf", "fl_r")):
            x = np.asarray(inputs[f"{src}{i}"][b], np.float32).reshape(C, L)
            m[f"{nm}{i}"] = np.ascontiguousarray(x).astype(NP_BF16)
        for nm in ("wq", "wk", "wv"):
            w = np.asarray(inputs[f"{nm}{i}"], np.float32)[:, sl]
            m[f"{nm}{i}"] = np.ascontiguousarray(w).astype(NP_BF16)
        wo = np.asarray(inputs[f"wo{i}"], np.float32)[sl, :]
        m[f"wo{i}"] = np.ascontiguousarray(wo).astype(NP_BF16)
        m[f"bq{i}"] = np.ascontiguousarray(
            np.asarray(inputs[f"bq{i}"], np.float32)[sl].reshape(128, 1))
        m[f"bk{i}"] = np.ascontiguousarray(
            np.asarray(inputs[f"bk{i}"], np.float32)[sl].reshape(128, 1))
        m[f"bv{i}"] = np.ascontiguousarray(
            np.asarray(inputs[f"bv{i}"], np.float32)[sl].reshape(1, 128))
    return m


def kernel(**inputs):
    global _NC, LAST_RESULTS
    if _NC is None:
        _NC = _build()
    in_maps = [_core_inputs(inputs, c) for c in range(N_CORES)]
    res = run_bass_kernel_spmd(
        _NC,
        in_maps,
        core_ids=list(range(N_CORES)),
        trace=bool(int(os.environ.get("KERNEL_TRACE", "0"))),
    )
    LAST_RESULTS = res
    outs = []
    for i in range(4):
        C, R = DIMS[i], RES[i]
        L = R * R
        acc = np.zeros((B, C, L), np.float32)
        if i in LAYERS:
            for c in range(N_CORES):
                acc[c // 4] += res.results[c][f"out{i}"]
            acc += np.asarray(inputs[f"bo{i}"], np.float32)[None, :, None]
        outs.append(acc.reshape(B, C, R, R))
    return tuple(outs)
